# revision 1
# baseline (speedup 1.0000x reference)
"""Causal self-attention (T=4096, C=2048, 16 heads) on 8 TRN2 NeuronCores.

Sharding: tensor-parallel over heads (2 heads/core) for QKV + attention,
then per-head AllToAlls redistribute the attention output to
token-parallel (512 tokens/core) for the output projection. No reduction
collective is needed: each core computes full output rows for its token
slice and the host concatenates.

All matmuls run bf16 (inputs converted to bf16 on the host, halving DMA
bytes; PSUM accumulation stays fp32). Scores are computed transposed
(keys on partitions, queries free) so softmax denominators come from a
ones-vector matmul, P@V needs no transposes, and causal masking is a
bf16 multiply with 4 precomputed diagonal mask tiles; upper-triangle
blocks are skipped entirely. Softmax normalization is deferred across
the AllToAll: each A2A shard carries 128 rows of unnormalized P@V plus a
129th row with the softmax denominators, and the receiving side divides
- this keeps the phase-2 tensor-engine stream free of serializing
reciprocal chains (which otherwise reset the PE clock-warmup state).
"""
import sys
import types

sys.path.insert(0, "/opt/trn_rl_repo")

import ml_dtypes
import numpy as np

from concourse import bacc, tile
import concourse.mybir as mybir
from concourse.bass_utils import run_bass_kernel_spmd

F32 = mybir.dt.float32
BF16 = mybir.dt.bfloat16
NP_BF16 = np.dtype(ml_dtypes.bfloat16)

T, C = 4096, 2048
H, D = 16, 128
W = 8                  # cores
HL = H // W            # heads per core (2)
CL = HL * D            # local attention-output columns (256)
KT = C // 128          # contraction tiles (16)
TC1 = 512              # phase-1 token chunk
NC1 = T // TC1         # 8
TC2 = 512              # phase-2/3 token chunk
NC2 = T // TC2         # 8
TL = T // W            # tokens per core for the projection (512)
SCALE = float(1.0 / np.sqrt(D))

TRACE = False          # test harness sets kernel.TRACE = True for profiling
LAST_RESULT = {}       # test harness reads exec_time_ns from here

_cache = {}


def _build():
    nc = bacc.Bacc("TRN2", target_bir_lowering=False, debug=False, num_devices=W)
    xT_d = nc.dram_tensor("xT", [C, T], BF16, kind="ExternalInput")
    wqkT_d = nc.dram_tensor("wqkT", [C, 2 * CL], BF16, kind="ExternalInput")
    wvT_d = nc.dram_tensor("wvT", [C, CL], BF16, kind="ExternalInput")
    wpT_d = nc.dram_tensor("wpT", [C, C], BF16, kind="ExternalInput")
    out_d = nc.dram_tensor("out", [TL, C], F32, kind="ExternalOutput")

    with tile.TileContext(nc) as tc:
        with tc.tile_pool(name="res", bufs=1) as res, \
             tc.tile_pool(name="dram", bufs=1, space="DRAM") as dram:
            # per-head A2A buffers (bf16): shard j = my token chunk j.
            # att buffers keep 128-row shards (alignment matters for A2A
            # bandwidth); softmax denominators ride separate tiny A2As.
            a2a_in = [dram.tile([W, 128, TC2], BF16, tag=f"a2a_in{h}",
                                name=f"a2a_in{h}") for h in range(HL)]
            a2a_out = [dram.tile([W, 128, TC2], BF16, tag=f"a2a_out{h}",
                                 name=f"a2a_out{h}") for h in range(HL)]
            d2a_in = [dram.tile([W, 1, TC2], BF16, tag=f"d2a_in{h}",
                                name=f"d2a_in{h}") for h in range(HL)]
            d2a_out = [dram.tile([W, 1, TC2], BF16, tag=f"d2a_out{h}",
                                 name=f"d2a_out{h}") for h in range(HL)]

            # resident q/k (transposed, [d, t]) and V ([s, d]), all bf16
            qT = [res.tile([128, T], BF16, tag=f"qT{h}", name=f"qT{h}")
                  for h in range(HL)]
            kT = [res.tile([128, T], BF16, tag=f"kT{h}", name=f"kT{h}")
                  for h in range(HL)]
            V = [res.tile([128, CL], BF16, tag=f"V{i}", name=f"V{i}")
                 for i in range(T // 128)]

            ones32 = res.tile([128, 1], F32, tag="ones32")
            nc.gpsimd.memset(ones32[:], 1.0)
            ones = res.tile([128, 1], BF16, tag="ones")
            nc.vector.tensor_copy(ones[:], ones32[:])

            # 4 diagonal causal masks (keep where t >= s within the tile):
            # mask dk applies to s-tile k = 4j + dk of query chunk j
            masks = []
            for dk in range(4):
                m32 = res.tile([128, TC2], F32, tag=f"m32_{dk}",
                               name=f"m32_{dk}")
                nc.gpsimd.memset(m32[:], 1.0)
                mb = res.tile([128, TC2], BF16, tag=f"mask{dk}",
                              name=f"mask{dk}")
                nc.vector.tensor_copy(mb[:], m32[:])
                nc.gpsimd.affine_select(
                    out=mb[:], in_=mb[:],
                    compare_op=mybir.AluOpType.is_ge,
                    fill=0.0,
                    base=-128 * dk,
                    channel_multiplier=-1,
                    pattern=[[1, TC2]],
                )
                masks.append(mb)

            # ---------------- phase 1: QKV projection (bf16) ----------------
            with tc.tile_pool(name="wpool", bufs=1) as wpool, \
                 tc.tile_pool(name="xpool", bufs=2) as xpool, \
                 tc.tile_pool(name="ps1", bufs=3, space="PSUM") as ps1:
                wqk = [[None] * 4 for _ in range(KT)]

                def load_wqk(k):
                    for m in range(4):
                        t_ = wpool.tile([128, 128], BF16,
                                        tag=f"wqk{k}_{m}", name=f"wqk{k}_{m}")
                        nc.sync.dma_start(
                            t_[:],
                            wqkT_d.ap()[k * 128:(k + 1) * 128,
                                        m * 128:(m + 1) * 128],
                        )
                        wqk[k][m] = t_

                for k in range(KT):
                    load_wqk(k)

                def load_x_chunk(j):
                    xt = []
                    for k in range(KT):
                        t_ = xpool.tile([128, TC1], BF16, tag=f"x{k}",
                                        name=f"x{j}_{k}")
                        nc.sync.dma_start(
                            t_[:],
                            xT_d.ap()[k * 128:(k + 1) * 128,
                                      j * TC1:(j + 1) * TC1],
                        )
                        xt.append(t_)
                    return xt

                xt0 = load_x_chunk(0)
                for k in range(1, KT):
                    load_wqk(k)
                wv = []
                for k in range(KT):
                    t_ = wpool.tile([128, CL], BF16, tag=f"wv{k}", name=f"wv{k}")
                    nc.sync.dma_start(
                        t_[:], wvT_d.ap()[k * 128:(k + 1) * 128, :])
                    wv.append(t_)

                for j in range(NC1):
                    xt = xt0 if j == 0 else load_x_chunk(j)
                    # qT/kT for both heads: out[d, t] accumulated over c
                    for m in range(4):
                        pq = ps1.tile([128, TC1], F32, tag="pqk")
                        for k in range(KT):
                            nc.tensor.matmul(pq[:], wqk[k][m][:], xt[k][:],
                                             start=(k == 0), stop=(k == KT - 1))
                        dest = qT[m] if m < HL else kT[m - HL]
                        nc.vector.tensor_copy(
                            dest[:, j * TC1:(j + 1) * TC1], pq[:])
                    # V: out[t, d] accumulated over c
                    for tt in range(TC1 // 128):
                        pv = ps1.tile([128, CL], F32, tag="pv")
                        for k in range(KT):
                            nc.tensor.matmul(
                                pv[:],
                                xt[k][:, tt * 128:(tt + 1) * 128],
                                wv[k][:],
                                start=(k == 0), stop=(k == KT - 1))
                        nc.scalar.copy(V[j * (TC1 // 128) + tt][:], pv[:])

            # ---------------- phases 2+3 pools ----------------
            with tc.tile_pool(name="ph2", bufs=6) as p2, \
                 tc.tile_pool(name="a2s", bufs=3) as a2s, \
                 tc.tile_pool(name="p3a", bufs=1) as p3a, \
                 tc.tile_pool(name="p3n", bufs=2) as p3n, \
                 tc.tile_pool(name="p3w", bufs=1) as p3w, \
                 tc.tile_pool(name="p3o", bufs=2) as p3o:
                # prefetch the full projection weight during phase 2:
                # these DMAs sit on the sync queue ahead of the att writes
                wp = []
                for oc in range(C // 512):
                    row = []
                    for kc in range(KT):
                        t_ = p3w.tile([128, 512], BF16, tag=f"wp{oc}_{kc}",
                                      name=f"wp{oc}_{kc}")
                        nc.sync.dma_start(
                            t_[:],
                            wpT_d.ap()[kc * 128:(kc + 1) * 128,
                                       oc * 512:(oc + 1) * 512],
                        )
                        row.append(t_)
                    wp.append(row)

                # ---------------- phase 2: attention (bf16) ----------------
                with tc.tile_pool(name="ps2s", bufs=3, space="PSUM") as ps2s, \
                     tc.tile_pool(name="ps2o", bufs=2, space="PSUM") as ps2o, \
                     tc.tile_pool(name="ps2d", bufs=1, space="PSUM") as ps2d:
                    for h in range(HL):
                        for j in range(NC2):
                            nk = (j + 1) * (TC2 // 128)  # causal s tiles
                            po = ps2o.tile([128, TC2], F32, tag="po")
                            pd = ps2d.tile([1, TC2], F32, tag="pd")
                            for k in range(nk):
                                ps = ps2s.tile([128, TC2], F32, tag="ps")
                                nc.tensor.matmul(
                                    ps[:],
                                    kT[h][:, k * 128:(k + 1) * 128],
                                    qT[h][:, j * TC2:(j + 1) * TC2],
                                    start=True, stop=True)
                                e = p2.tile([128, TC2], BF16, tag="e")
                                nc.scalar.activation(
                                    e[:], ps[:],
                                    mybir.ActivationFunctionType.Exp,
                                    scale=SCALE)
                                dk = k - 4 * j
                                if dk >= 0:
                                    # diagonal tile: zero out s > t entries
                                    nc.vector.tensor_mul(e[:], e[:],
                                                         masks[dk][:])
                                nc.tensor.matmul(pd[:], ones[:], e[:],
                                                 start=(k == 0),
                                                 stop=(k == nk - 1))
                                nc.tensor.matmul(
                                    po[:],
                                    V[k][:, h * 128:(h + 1) * 128],
                                    e[:],
                                    start=(k == 0), stop=(k == nk - 1))
                            att = a2s.tile([128, TC2], BF16, tag="att")
                            nc.scalar.copy(att[:], po[:])
                            den = a2s.tile([1, TC2], BF16, tag="den")
                            nc.scalar.copy(den[:], pd[:])
                            nc.sync.dma_start(a2a_in[h][j, :, :], att[:])
                            nc.sync.dma_start(d2a_in[h][j, 0, :], den[:])
                        # fire this head's A2As as soon as its chunks are
                        # written; head 0's collectives overlap head 1
                        nc.gpsimd.collective_compute(
                            "AllToAll",
                            mybir.AluOpType.bypass,
                            ins=[a2a_in[h].opt()],
                            outs=[a2a_out[h].opt()],
                            replica_groups=[list(range(W))],
                        )
                        nc.gpsimd.collective_compute(
                            "AllToAll",
                            mybir.AluOpType.bypass,
                            ins=[d2a_in[h].opt()],
                            outs=[d2a_out[h].opt()],
                            replica_groups=[list(range(W))],
                        )

                # ---------------- phase 3: output projection (bf16) ----------------
                # load unnormalized attention + denominators, divide locally.
                # Per-head denominators arrive with that head's tiny A2A, so
                # head 0's normalization overlaps head 1's attention compute.
                attn = [None] * KT
                for h in range(HL):
                    den8 = p3n.tile([W, TL], BF16, tag=f"den8_{h}",
                                    name=f"den8_{h}")
                    for i in range(W):
                        kc = i * HL + h
                        t_ = p3a.tile([128, TL], BF16, tag=f"at{kc}",
                                      name=f"at{kc}")
                        nc.sync.dma_start(t_[:], a2a_out[h][i, :, :])
                        nc.sync.dma_start(den8[i:i + 1, :],
                                          d2a_out[h][i, 0, :])
                        attn[kc] = t_
                    den32 = p3n.tile([W, TL], F32, tag=f"den32_{h}",
                                     name=f"den32_{h}")
                    nc.vector.tensor_copy(den32[:], den8[:])
                    rec32 = p3n.tile([W, TL], F32, tag=f"rec32_{h}",
                                     name=f"rec32_{h}")
                    nc.vector.reciprocal(rec32[:], den32[:])
                    for i in range(W):
                        kc = i * HL + h
                        # row i -> partition 0 (DMA), then broadcast to 128;
                        # muls on gpsimd to keep DVE free for phase-2 masks
                        r1 = p3n.tile([1, TL], F32, tag="r1", bufs=4,
                                      name=f"r1_{kc}")
                        nc.sync.dma_start(r1[:], rec32[i:i + 1, :])
                        r128 = p3n.tile([128, TL], F32, tag="r128", bufs=3,
                                        name=f"r128_{kc}")
                        nc.gpsimd.partition_broadcast(r128[:], r1[:])
                        nc.gpsimd.tensor_mul(attn[kc][:], attn[kc][:], r128[:])
                # split accumulation: even kc (head-0 sourced, available
                # before the second A2A) first, so the tensor engine works
                # through the A2A/normalize latency of the odd tiles
                with tc.tile_pool(name="ps3", bufs=1, space="PSUM") as ps3:
                    for og in range(2):
                        po3s = {}
                        for oc in (2 * og, 2 * og + 1):
                            for tt in range(TL // 128):
                                po3 = ps3.tile([128, 512], F32,
                                               tag=f"po3_{oc % 2}_{tt}",
                                               name=f"po3_{oc}_{tt}")
                                po3s[(oc, tt)] = po3
                                for kc in range(0, KT, 2):
                                    nc.tensor.matmul(
                                        po3[:],
                                        attn[kc][:, tt * 128:(tt + 1) * 128],
                                        wp[oc][kc][:],
                                        start=(kc == 0), stop=False)
                        for oc in (2 * og, 2 * og + 1):
                            for tt in range(TL // 128):
                                po3 = po3s[(oc, tt)]
                                for kc in range(1, KT, 2):
                                    nc.tensor.matmul(
                                        po3[:],
                                        attn[kc][:, tt * 128:(tt + 1) * 128],
                                        wp[oc][kc][:],
                                        start=False, stop=(kc == KT - 1))
                                ob = p3o.tile([128, 512], F32, tag="ob")
                                nc.scalar.copy(ob[:], po3[:])
                                nc.sync.dma_start(
                                    out_d.ap()[tt * 128:(tt + 1) * 128,
                                               oc * 512:(oc + 1) * 512],
                                    ob[:])

    nc.compile()
    return nc


def _maybe_install_trace_hook():
    try:
        import antenv
        from trn_agent_boot.trn_boot import _ntff_profile_via_ctypes
        hook = _ntff_profile_via_ctypes("/opt/axon/libaxon_pjrt.so")
        mod = types.ModuleType("antenv.axon_hooks")
        mod.get_axon_ntff_profile_hook = lambda: hook
        mod.set_axon_ntff_profile_hook = lambda h: None
        sys.modules["antenv.axon_hooks"] = mod
        antenv.axon_hooks = mod
        return True
    except Exception:
        return False


def kernel(x, w_attn, w_proj):
    x = np.ascontiguousarray(x, dtype=np.float32)
    w_attn = np.ascontiguousarray(w_attn, dtype=np.float32)
    w_proj = np.ascontiguousarray(w_proj, dtype=np.float32)

    if "nc" not in _cache:
        _cache["nc"] = _build()
    nc = _cache["nc"]

    xT = np.ascontiguousarray(x.T).astype(NP_BF16)
    wpT = np.ascontiguousarray(w_proj.T).astype(NP_BF16)
    in_maps = []
    for c in range(W):
        r0 = CL * c
        wqk = np.concatenate(
            [w_attn[r0:r0 + CL], w_attn[C + r0:C + r0 + CL]], axis=0)
        wqkT = np.ascontiguousarray(wqk.T).astype(NP_BF16)
        wvT = np.ascontiguousarray(
            w_attn[2 * C + r0:2 * C + r0 + CL].T).astype(NP_BF16)
        in_maps.append({"xT": xT, "wqkT": wqkT, "wvT": wvT, "wpT": wpT})

    trace = TRACE and _maybe_install_trace_hook()
    res = run_bass_kernel_spmd(nc, in_maps, list(range(W)), trace=trace)
    LAST_RESULT["exec_time_ns"] = res.exec_time_ns

    return np.concatenate([res.results[c]["out"] for c in range(W)], axis=0)



# revision 3
# speedup vs baseline: 1.3143x; 1.3143x over previous
"""Causal self-attention (T=4096, C=2048, 16 heads) on 8 TRN2 NeuronCores.

Sharding: tensor-parallel over heads (2 heads/core) for QKV + attention,
then per-head AllToAlls redistribute the attention output to
token-parallel (512 tokens/core) for the output projection. No reduction
collective is needed: each core computes full output rows for its token
slice and the host concatenates.

All matmuls run bf16 (inputs converted to bf16 on the host, halving DMA
bytes; PSUM accumulation stays fp32). Scores are computed transposed
(keys on partitions, queries free) so P@V needs no transposes; causal
masking is a bf16 multiply with 4 precomputed diagonal mask tiles and
upper-triangle blocks are skipped entirely.

vs. the previous revision:
- All bulk HBM loads are single batched dma_starts (3-D access patterns)
  instead of per-128x512-tile dispatches: the SP sequencer costs ~565ns
  per dma_start, and ~150 serialized dispatches used to delay the first
  matmul by ~53us.
- Exp runs on [128,1024] pairs of score tiles (two PSUM banks) so the
  scalar engine's per-instruction overhead doesn't gate the tensor
  engine's score+PV cadence.
- Softmax denominators come from DVE-accumulated exp sums (one
  [128,512] bf16 running sum per chunk) plus a single ones-vector
  matmul per chunk - the per-k-tile ones-matmuls used to burn ~60us of
  tensor-engine time.
- Normalization happens on the SOURCE side of the AllToAll (reciprocal
  + partition-broadcast + DVE multiply, all off the tensor path), so
  phase 3 is pure DMA + matmul and the denominator A2As disappear.
"""
import sys
import types

sys.path.insert(0, "/opt/trn_rl_repo")

import ml_dtypes
import numpy as np

from concourse import bacc, tile
import concourse.mybir as mybir
from concourse.bass_utils import run_bass_kernel_spmd

F32 = mybir.dt.float32
BF16 = mybir.dt.bfloat16
NP_BF16 = np.dtype(ml_dtypes.bfloat16)

T, C = 4096, 2048
H, D = 16, 128
W = 8                  # cores
HL = H // W            # heads per core (2)
CL = HL * D            # local attention-output columns (256)
KT = C // 128          # contraction tiles (16)
TC = 512               # token chunk (phases 1-3)
NC = T // TC           # 8
TL = T // W            # tokens per core for the projection (512)
SCALE = float(1.0 / np.sqrt(D))

TRACE = False          # test harness sets kernel.TRACE = True for profiling
LAST_RESULT = {}       # test harness reads exec_time_ns from here

_cache = {}


def _build():
    nc = bacc.Bacc("TRN2", target_bir_lowering=False, debug=False, num_devices=W)
    xT_d = nc.dram_tensor("xT", [C, T], BF16, kind="ExternalInput")
    wqkT_d = nc.dram_tensor("wqkT", [C, 2 * CL], BF16, kind="ExternalInput")
    wvT_d = nc.dram_tensor("wvT", [C, CL], BF16, kind="ExternalInput")
    wpT_d = nc.dram_tensor("wpT", [C, C], BF16, kind="ExternalInput")
    out_d = nc.dram_tensor("out", [TL, C], F32, kind="ExternalOutput")

    with tile.TileContext(nc) as tc:
        with tc.tile_pool(name="res", bufs=1) as res, \
             tc.tile_pool(name="dram", bufs=1, space="DRAM") as dram:
            # per-head A2A buffers (bf16): shard j = my token chunk j.
            a2a_in = [dram.tile([W, 128, TC], BF16, tag=f"a2a_in{h}",
                                name=f"a2a_in{h}") for h in range(HL)]
            a2a_out = [dram.tile([W, 128, TC], BF16, tag=f"a2a_out{h}",
                                 name=f"a2a_out{h}") for h in range(HL)]

            # resident q/k (transposed, [d, t]) and V ([s, d]), all bf16
            qT = [res.tile([128, T], BF16, tag=f"qT{h}", name=f"qT{h}")
                  for h in range(HL)]
            kT = [res.tile([128, T], BF16, tag=f"kT{h}", name=f"kT{h}")
                  for h in range(HL)]
            V = [res.tile([128, CL], BF16, tag=f"V{i}", name=f"V{i}")
                 for i in range(T // 128)]

            ones32 = res.tile([128, 1], F32, tag="ones32")
            nc.gpsimd.memset(ones32[:], 1.0)
            ones = res.tile([128, 1], BF16, tag="ones")
            nc.vector.tensor_copy(ones[:], ones32[:])

            # 4 diagonal causal masks (keep where t >= s within the tile):
            # mask dk applies to s-tile k = 4j + dk of query chunk j
            masks = []
            for dk in range(4):
                m32 = res.tile([128, TC], F32, tag=f"m32_{dk}",
                               name=f"m32_{dk}")
                nc.gpsimd.memset(m32[:], 1.0)
                mb = res.tile([128, TC], BF16, tag=f"mask{dk}",
                              name=f"mask{dk}")
                nc.vector.tensor_copy(mb[:], m32[:])
                nc.gpsimd.affine_select(
                    out=mb[:], in_=mb[:],
                    compare_op=mybir.AluOpType.is_ge,
                    fill=0.0,
                    base=-128 * dk,
                    channel_multiplier=-1,
                    pattern=[[1, TC]],
                )
                masks.append(mb)

            # ---------------- phase 1: QKV projection (bf16) ----------------
            with tc.tile_pool(name="wpool", bufs=1) as wpool, \
                 tc.tile_pool(name="xpool", bufs=2) as xpool, \
                 tc.tile_pool(name="ps1", bufs=3, space="PSUM") as ps1:
                # single batched loads: [k-tile, partition, col] -> one
                # dma_start each (inner 1KB descriptor runs)
                wqk = wpool.tile([128, KT * 512], BF16, tag="wqk", name="wqk")
                nc.sync.dma_start(
                    wqk[:].rearrange("p (k m) -> p k m", k=KT, m=512),
                    wqkT_d.ap().rearrange("(k p) m -> p k m", k=KT, p=128),
                )

                def load_x_chunk(j):
                    xt = xpool.tile([128, KT * TC], BF16, tag="x",
                                    name=f"x{j}")
                    nc.sync.dma_start(
                        xt[:].rearrange("p (k t) -> p k t", k=KT, t=TC),
                        xT_d.ap()[:, j * TC:(j + 1) * TC].rearrange(
                            "(k p) t -> p k t", k=KT, p=128),
                    )
                    return xt

                xt0 = load_x_chunk(0)
                wv = wpool.tile([128, KT * CL], BF16, tag="wv", name="wv")
                nc.sync.dma_start(
                    wv[:].rearrange("p (k d) -> p k d", k=KT, d=CL),
                    wvT_d.ap().rearrange("(k p) d -> p k d", k=KT, p=128),
                )

                for j in range(NC):
                    xt = xt0 if j == 0 else load_x_chunk(j)
                    # qT/kT for both heads: out[d, t] accumulated over c
                    for m in range(4):
                        pq = ps1.tile([128, TC], F32, tag="pqk")
                        for k in range(KT):
                            nc.tensor.matmul(
                                pq[:],
                                wqk[:, k * 512 + m * 128:
                                    k * 512 + (m + 1) * 128],
                                xt[:, k * TC:(k + 1) * TC],
                                start=(k == 0), stop=(k == KT - 1))
                        dest = qT[m] if m < HL else kT[m - HL]
                        nc.vector.tensor_copy(
                            dest[:, j * TC:(j + 1) * TC], pq[:])
                    # V: out[t, d] accumulated over c
                    for tt in range(TC // 128):
                        pv = ps1.tile([128, CL], F32, tag="pv")
                        for k in range(KT):
                            nc.tensor.matmul(
                                pv[:],
                                xt[:, k * TC + tt * 128:
                                   k * TC + (tt + 1) * 128],
                                wv[:, k * CL:(k + 1) * CL],
                                start=(k == 0), stop=(k == KT - 1))
                        nc.scalar.copy(V[j * (TC // 128) + tt][:], pv[:])

            # ---------------- phases 2+3 pools ----------------
            with tc.tile_pool(name="ph2", bufs=3) as p2, \
                 tc.tile_pool(name="a2s", bufs=3) as a2s, \
                 tc.tile_pool(name="p3a", bufs=1) as p3a, \
                 tc.tile_pool(name="p2n", bufs=2) as p2n, \
                 tc.tile_pool(name="p3w", bufs=1) as p3w, \
                 tc.tile_pool(name="p3o", bufs=2) as p3o:
                # prefetch the full projection weight during phase 2 with a
                # single dma_start ([k, partition, outcol] pattern)
                wp = p3w.tile([128, KT * C], BF16, tag="wp", name="wp")
                nc.sync.dma_start(
                    wp[:].rearrange("p (k o) -> p k o", k=KT, o=C),
                    wpT_d.ap().rearrange("(k p) o -> p k o", k=KT, p=128),
                )

                attn_all = [None] * HL

                # ---------------- phase 2: attention (bf16) ----------------
                with tc.tile_pool(name="ps2s", bufs=2, space="PSUM") as ps2s, \
                     tc.tile_pool(name="ps2o", bufs=2, space="PSUM") as ps2o, \
                     tc.tile_pool(name="ps2d", bufs=1, space="PSUM") as ps2d:
                    pending = None
                    for h in range(HL):
                        for j in range(NC):
                            npairs = (j + 1) * (TC // 128) // 2
                            po = ps2o.tile([128, TC], F32, tag="po")
                            esum = p2n.tile([128, TC], BF16, tag="esum")
                            for p in range(npairs):
                                k0, k1 = 2 * p, 2 * p + 1
                                ps = ps2s.tile([128, 1024], F32, tag="ps")
                                nc.tensor.matmul(
                                    ps[:, 0:512],
                                    kT[h][:, k0 * 128:(k0 + 1) * 128],
                                    qT[h][:, j * TC:(j + 1) * TC],
                                    start=True, stop=True)
                                nc.tensor.matmul(
                                    ps[:, 512:1024],
                                    kT[h][:, k1 * 128:(k1 + 1) * 128],
                                    qT[h][:, j * TC:(j + 1) * TC],
                                    start=True, stop=True)
                                if p == 1 and pending is not None:
                                    # previous chunk's denominator matmul +
                                    # normalize, deferred so the DVE exp-sum
                                    # has a pair of score matmuls to hide
                                    # behind
                                    pending()
                                    pending = None
                                e2 = p2.tile([128, 1024], BF16, tag="e")
                                nc.scalar.activation(
                                    e2[:], ps[:],
                                    mybir.ActivationFunctionType.Exp,
                                    scale=SCALE)
                                for half in range(2):
                                    dk = 2 * p + half - 4 * j
                                    if dk >= 0:
                                        # diagonal tile: zero s > t entries
                                        sl = e2[:, half * 512:
                                                (half + 1) * 512]
                                        nc.vector.tensor_mul(
                                            sl, sl, masks[dk][:])
                                if p == 0:
                                    nc.vector.tensor_add(
                                        esum[:], e2[:, 0:512], e2[:, 512:1024])
                                else:
                                    nc.vector.tensor_add(
                                        esum[:], esum[:], e2[:, 0:512])
                                    nc.vector.tensor_add(
                                        esum[:], esum[:], e2[:, 512:1024])
                                nc.tensor.matmul(
                                    po[:],
                                    V[k0][:, h * 128:(h + 1) * 128],
                                    e2[:, 0:512],
                                    start=(p == 0), stop=False)
                                nc.tensor.matmul(
                                    po[:],
                                    V[k1][:, h * 128:(h + 1) * 128],
                                    e2[:, 512:1024],
                                    start=False, stop=(p == npairs - 1))

                            def make_norm(h=h, j=j, po=po, esum=esum):
                                def norm():
                                    pd = ps2d.tile([1, TC], F32, tag="pd")
                                    nc.tensor.matmul(pd[:], ones[:], esum[:],
                                                     start=True, stop=True)
                                    rec = p2n.tile([1, TC], F32, tag="rec")
                                    nc.vector.reciprocal(rec[:], pd[:])
                                    r128 = p2n.tile([128, TC], F32,
                                                    tag="r128")
                                    nc.gpsimd.partition_broadcast(
                                        r128[:], rec[:])
                                    att = a2s.tile([128, TC], BF16,
                                                   tag="att")
                                    nc.vector.tensor_mul(att[:], po[:],
                                                         r128[:])
                                    nc.sync.dma_start(a2a_in[h][j, :, :],
                                                      att[:])
                                return norm

                            pending = make_norm()
                        # flush the last chunk's normalize, then fire this
                        # head's A2A; head 0's collective overlaps head 1
                        pending()
                        pending = None
                        nc.gpsimd.collective_compute(
                            "AllToAll",
                            mybir.AluOpType.bypass,
                            ins=[a2a_in[h].opt()],
                            outs=[a2a_out[h].opt()],
                            replica_groups=[list(range(W))],
                        )
                        # batched read-back of this head's shards (16 heads
                        # x my 512 tokens land as 8 [128, 512] blocks)
                        attn_all[h] = p3a.tile([128, W * TC], BF16,
                                               tag=f"attn{h}",
                                               name=f"attn{h}")
                        nc.sync.dma_start(
                            attn_all[h][:].rearrange(
                                "p (i t) -> p i t", i=W, t=TC),
                            a2a_out[h][:, :, :].rearrange("i p t -> p i t"),
                        )

                # ---------------- phase 3: output projection (bf16) --------
                # attn tile for kc = i*HL + h is attn_all[h][:, i*512:...].
                # Even kc (head-0 sourced, available before the second A2A)
                # accumulates first, so the tensor engine works through the
                # second A2A's latency.
                with tc.tile_pool(name="ps3", bufs=1, space="PSUM") as ps3:
                    for og in range(2):
                        po3s = {}
                        for oc in (2 * og, 2 * og + 1):
                            for tt in range(TL // 128):
                                po3 = ps3.tile([128, 512], F32,
                                               tag=f"po3_{oc % 2}_{tt}",
                                               name=f"po3_{oc}_{tt}")
                                po3s[(oc, tt)] = po3
                                for i in range(W):
                                    kc = i * HL
                                    nc.tensor.matmul(
                                        po3[:],
                                        attn_all[0][:, i * TC + tt * 128:
                                                    i * TC + (tt + 1) * 128],
                                        wp[:, kc * C + oc * 512:
                                           kc * C + (oc + 1) * 512],
                                        start=(i == 0), stop=False)
                        for oc in (2 * og, 2 * og + 1):
                            for tt in range(TL // 128):
                                po3 = po3s[(oc, tt)]
                                for i in range(W):
                                    kc = i * HL + 1
                                    nc.tensor.matmul(
                                        po3[:],
                                        attn_all[1][:, i * TC + tt * 128:
                                                    i * TC + (tt + 1) * 128],
                                        wp[:, kc * C + oc * 512:
                                           kc * C + (oc + 1) * 512],
                                        start=False, stop=(i == W - 1))
                                ob = p3o.tile([128, 512], F32, tag="ob")
                                nc.scalar.copy(ob[:], po3[:])
                                nc.sync.dma_start(
                                    out_d.ap()[tt * 128:(tt + 1) * 128,
                                               oc * 512:(oc + 1) * 512],
                                    ob[:])

    nc.compile()
    return nc


def _maybe_install_trace_hook():
    try:
        import antenv
        from trn_agent_boot.trn_boot import _ntff_profile_via_ctypes
        hook = _ntff_profile_via_ctypes("/opt/axon/libaxon_pjrt.so")
        mod = types.ModuleType("antenv.axon_hooks")
        mod.get_axon_ntff_profile_hook = lambda: hook
        mod.set_axon_ntff_profile_hook = lambda h: None
        sys.modules["antenv.axon_hooks"] = mod
        antenv.axon_hooks = mod
        return True
    except Exception:
        return False


def kernel(x, w_attn, w_proj):
    x = np.ascontiguousarray(x, dtype=np.float32)
    w_attn = np.ascontiguousarray(w_attn, dtype=np.float32)
    w_proj = np.ascontiguousarray(w_proj, dtype=np.float32)

    if "nc" not in _cache:
        _cache["nc"] = _build()
    nc = _cache["nc"]

    xT = np.ascontiguousarray(x.T).astype(NP_BF16)
    wpT = np.ascontiguousarray(w_proj.T).astype(NP_BF16)
    in_maps = []
    for c in range(W):
        r0 = CL * c
        wqk = np.concatenate(
            [w_attn[r0:r0 + CL], w_attn[C + r0:C + r0 + CL]], axis=0)
        wqkT = np.ascontiguousarray(wqk.T).astype(NP_BF16)
        wvT = np.ascontiguousarray(
            w_attn[2 * C + r0:2 * C + r0 + CL].T).astype(NP_BF16)
        in_maps.append({"xT": xT, "wqkT": wqkT, "wvT": wvT, "wpT": wpT})

    trace = TRACE and _maybe_install_trace_hook()
    res = run_bass_kernel_spmd(nc, in_maps, list(range(W)), trace=trace)
    LAST_RESULT["exec_time_ns"] = res.exec_time_ns

    return np.concatenate([res.results[c]["out"] for c in range(W)], axis=0)
